# revision 4
# baseline (speedup 1.0000x reference)
"""ConvSquare Trainium2 kernel.

Math: out = conv2d_3x3(x * p, weight) + bias, stride 1, pad 1, where
p = (a*alpha + b)*alpha + c on the zero-padded alpha field. (x is
zero-padded, so border window positions contribute 0 regardless of p.)

Sharding: 8 cores = batch(4) x row-half(2). Each core computes a
[O=64, 64, 128] output slab from a zero-padded [C=64, 67, 130] slab
(67th row all-zero, backing the +1-row shifted copy).

Device pipeline per core (bf16 datapath, f32 accumulate/output):
  - x loaded twice from HBM: partitions 0-63 = rows 0-65, partitions
    64-127 = rows 1-66 (the +1-row shift baked in at load time - no
    SBUF->SBUF shift copy).
  - p field (host-precomputed tiny poly, 0.001% of FLOPs) broadcast
    from HBM to both halves with the same shift.
  - One DVE tensor_mul per chunk produces y AND shifted-y together
    ([128, n] op costs the same as [64, n]; bf16 gets the 2x mode).
  - 6 matmuls per 512-col output chunk: 3 paired taps (k=0,1) over the
    128-partition tile + 3 singles (k=2) on the lower half.
  - ACT engine adds bias while copying PSUM->SBUF staging; grouped
    SBUF->HBM stores.
  - A few tiny warm-up matmuls ramp the PE clock before real work.
"""

import sys

import numpy as np

sys.path.insert(0, "/opt/trn_rl_repo")

import ml_dtypes

import concourse.bass as bass
import concourse.mybir as mybir
from concourse.bass_utils import run_bass_kernel_spmd
from concourse.tile import TileContext

F32 = mybir.dt.float32
BF16 = mybir.dt.bfloat16

B, C, O, H, W = 4, 64, 64, 128, 128
HS = 64  # output rows per core
RP = HS + 2  # padded input rows (66)
WP = W + 2  # padded cols (130)
FREE = RP * WP  # 8580
FREE2 = (RP + 1) * WP  # 8710: one extra all-zero row for the shifted half
NCHUNK = 16  # matmul chunks (4 out rows each)
MM_N = 4 * W  # 512
EW_CH = 6  # elementwise chunks (11 rows each)
EW_N = FREE // EW_CH  # 1430
N_WARM = 7
STORE_GROUPS = [(0, 6), (6, 10), (10, 13), (13, 15), (15, 16)]

_cache: dict = {}


def _program() -> bass.Bass:
    from concourse.bacc import Bacc

    nc = Bacc()
    x_h = nc.dram_tensor("x", [C, FREE2], BF16, kind="ExternalInput")
    m_h = nc.dram_tensor("m", [1, FREE2], BF16, kind="ExternalInput")
    w_h = nc.dram_tensor("w", [128, 384], BF16, kind="ExternalInput")
    bias_h = nc.dram_tensor("bias", [O, 1], F32, kind="ExternalInput")
    out_h = nc.dram_tensor("out", [O, HS * W], F32, kind="ExternalOutput")

    def bcast_ap(base, offset, n):
        return bass.AP(tensor=base.tensor, offset=offset, ap=[[0, C], [1, n]])

    with TileContext(nc) as tc:
        with (
            tc.tile_pool(name="const", bufs=1) as cpool,
            tc.tile_pool(name="work", bufs=1) as wpool,
            tc.tile_pool(name="psum", bufs=4, space="PSUM") as ppool,
        ):
            # PE warm-up: tiny matmuls on memset tiles, queued ahead of the
            # real ones so the clock is ramped when data arrives.
            wrm_w = cpool.tile([1, 1], BF16)
            wrm_r = cpool.tile([1, MM_N], BF16)
            nc.vector.memset(wrm_w[:, :], 0.0)
            nc.vector.memset(wrm_r[:, :], 0.0)
            for _ in range(N_WARM):
                pw = ppool.tile([O, MM_N], F32)
                nc.tensor.matmul(
                    pw[0:1, :], wrm_w[:, :], wrm_r[:, :], start=True, stop=True
                )

            wt = cpool.tile([128, 384], BF16)
            nc.sync.dma_start(out=wt[:, :], in_=w_h[:, :])
            bt = cpool.tile([O, 1], F32)
            nc.sync.dma_start(out=bt[:, :], in_=bias_h[:, :])

            xt = wpool.tile([128, FREE], BF16)
            pb = wpool.tile([128, FREE], BF16)
            yt = wpool.tile([128, FREE], BF16)
            st = wpool.tile([O, HS * W], F32)

            for j in range(EW_CH):
                sl = slice(j * EW_N, (j + 1) * EW_N)
                su = slice(j * EW_N + WP, (j + 1) * EW_N + WP)
                nc.sync.dma_start(out=xt[0:64, sl], in_=x_h[:, sl])
                nc.sync.dma_start(out=xt[64:128, sl], in_=x_h[:, su])
                nc.gpsimd.dma_start(
                    out=pb[0:64, sl], in_=bcast_ap(m_h[:, :], j * EW_N, EW_N)
                )
                nc.gpsimd.dma_start(
                    out=pb[64:128, sl], in_=bcast_ap(m_h[:, :], j * EW_N + WP, EW_N)
                )
                nc.vector.tensor_mul(yt[:, sl], pb[:, sl], xt[:, sl])

            y3 = yt[:].rearrange("p (r c) -> p r c", r=RP)
            for i in range(NCHUNK):
                ps = ppool.tile([O, MM_N], F32)
                p3 = ps[:].rearrange("p (r c) -> p r c", r=4)
                for l in range(3):
                    # singles: tap k=2, lower half only
                    nc.tensor.matmul(
                        p3,
                        wt[0:64, 192 + 64 * l : 192 + 64 * l + 64],
                        y3[0:64, 4 * i + 2 : 4 * i + 6, l : l + W],
                        start=(l == 0),
                        stop=False,
                    )
                for l in range(3):
                    # paired taps k=0 (lower half) + k=1 (shifted half)
                    nc.tensor.matmul(
                        p3,
                        wt[0:128, 64 * l : 64 * l + 64],
                        y3[0:128, 4 * i : 4 * i + 4, l : l + W],
                        start=False,
                        stop=(l == 2),
                    )
                # bias-add while copying PSUM -> SBUF staging (ACT engine)
                nc.scalar.add(st[:, 512 * i : 512 * i + 512], ps[:, :], bt[:, 0:1])
            for g0, g1 in STORE_GROUPS:
                nc.scalar.dma_start(
                    out=out_h[:, 512 * g0 : 512 * g1], in_=st[:, 512 * g0 : 512 * g1]
                )
    return nc


def _pack_weights(wt):
    """[O,C,3,3] -> [128, 384] bf16: cols l*64+o rows c|c = taps (0,l)|(1,l);
    cols 192+l*64+o rows c (lower 64) = tap (2,l)."""
    wk = wt.transpose(1, 2, 3, 0)  # [c, k, l, o]
    pair = np.concatenate([wk[:, 0], wk[:, 1]], axis=0).reshape(128, 192)
    single = wk[:, 2].reshape(64, 192)
    out = np.zeros((128, 384), np.float32)
    out[:, :192] = pair
    out[:64, 192:] = single
    return np.ascontiguousarray(out.astype(ml_dtypes.bfloat16))


def kernel(inputs, alpha, weight, bias, a, b, c):
    x = np.asarray(inputs, np.float32)
    al = np.asarray(alpha, np.float32)
    wt = np.asarray(weight, np.float32)
    bs = np.asarray(bias, np.float32)
    av, bv, cv = float(a), float(b), float(c)

    if "nc" not in _cache:
        nc_new = _program()
        nc_new.finalize()
        _cache["nc"] = nc_new
    nc = _cache["nc"]

    w_packed = _pack_weights(wt)
    b_packed = np.ascontiguousarray(bs.reshape(O, 1))

    in_maps = []
    for core in range(8):
        b_idx, hh = divmod(core, 2)
        r0 = hh * HS - 1  # global row of padded row 0
        xs = np.zeros((C, RP + 1, WP), np.float32)
        als = np.zeros((1, RP + 1, WP), np.float32)
        lo = max(0, r0)
        hi = min(H, r0 + RP)
        xs[:, lo - r0 : hi - r0, 1 : 1 + W] = x[b_idx, :, lo:hi, :]
        als[:, lo - r0 : hi - r0, 1 : 1 + W] = al[b_idx, :, lo:hi, :]
        # p = poly(alpha) on the padded field (p=c at padding; x=0 there)
        m = (av * als + bv) * als + cv
        in_maps.append(
            {
                "x": np.ascontiguousarray(
                    xs.reshape(C, FREE2).astype(ml_dtypes.bfloat16)
                ),
                "m": np.ascontiguousarray(
                    m.reshape(1, FREE2).astype(ml_dtypes.bfloat16)
                ),
                "w": w_packed,
                "bias": b_packed,
            }
        )

    res = run_bass_kernel_spmd(nc, in_maps, list(range(8)))

    out = np.empty((B, O, H, W), np.float32)
    for core in range(8):
        b_idx, hh = divmod(core, 2)
        out[b_idx, :, hh * HS : (hh + 1) * HS, :] = res.results[core]["out"].reshape(
            O, HS, W
        )
    return out


# revision 11
# speedup vs baseline: 1.0081x; 1.0081x over previous
"""ConvSquare Trainium2 kernel.

Math: out = conv2d_3x3(x * p, weight) + bias, stride 1, pad 1, where
p = (a*alpha + b)*alpha + c on the zero-padded alpha field. (x is
zero-padded, so border window positions contribute 0 regardless of p.)

Sharding: 8 cores = batch(4) x row-half(2). Each core computes a
[O=64, 64, 128] output slab from a zero-padded [C=64, 67, 130] slab
(67th row all-zero, backing the +1-row shifted copy).

Device pipeline per core (bf16 datapath, f32 accumulate/output):
  - x loaded twice from HBM: partitions 0-63 = rows 0-65, partitions
    64-127 = rows 1-66 (the +1-row shift baked in at load time - no
    SBUF->SBUF shift copy).
  - p field (host-precomputed tiny poly, 0.001% of FLOPs) broadcast
    from HBM to both halves with the same shift.
  - One DVE tensor_mul per chunk produces y AND shifted-y together
    ([128, n] op costs the same as [64, n]; bf16 gets the 2x mode).
  - 6 matmuls per 512-col output chunk: 3 paired taps (k=0,1) over the
    128-partition tile + 3 singles (k=2) on the lower half.
  - ACT engine adds bias while copying PSUM->SBUF staging; grouped
    SBUF->HBM stores.
  - A few tiny warm-up matmuls ramp the PE clock before real work.
"""

import sys

import numpy as np

sys.path.insert(0, "/opt/trn_rl_repo")

import ml_dtypes

import concourse.bass as bass
import concourse.mybir as mybir
from concourse.bass_utils import run_bass_kernel_spmd
from concourse.tile import TileContext

F32 = mybir.dt.float32
BF16 = mybir.dt.bfloat16

B, C, O, H, W = 4, 64, 64, 128, 128
HS = 64  # output rows per core
RP = HS + 2  # padded input rows (66)
WP = W + 2  # padded cols (130)
FREE = RP * WP  # 8580
FREE2 = (RP + 1) * WP  # 8710: one extra all-zero row for the shifted half
NCHUNK = 16  # matmul chunks (4 out rows each)
MM_N = 4 * W  # 512
# elementwise chunk edges (cols): small first chunk so PE starts early
EW_EDGES = [0, 780, 2080, 3380, 4680, 5980, 7280, 8580]
N_WARM = 6
STORE_GROUPS = [(0, 6), (6, 10), (10, 13), (13, 15), (15, 16)]

_cache: dict = {}


def _program() -> bass.Bass:
    from concourse.bacc import Bacc

    nc = Bacc()
    x_h = nc.dram_tensor("x", [C, FREE2], BF16, kind="ExternalInput")
    m_h = nc.dram_tensor("m", [1, FREE2], BF16, kind="ExternalInput")
    w_h = nc.dram_tensor("w", [128, 384], BF16, kind="ExternalInput")
    bias_h = nc.dram_tensor("bias", [O, 1], F32, kind="ExternalInput")
    bb_h = nc.dram_tensor("bb", [1, O], BF16, kind="ExternalInput")
    out_h = nc.dram_tensor("out", [O, HS * W], F32, kind="ExternalOutput")

    def bcast_ap(base, offset, n):
        return bass.AP(tensor=base.tensor, offset=offset, ap=[[0, C], [1, n]])

    with TileContext(nc) as tc:
        with (
            tc.tile_pool(name="const", bufs=1) as cpool,
            tc.tile_pool(name="work", bufs=1) as wpool,
            tc.tile_pool(name="psum", bufs=4, space="PSUM") as ppool,
        ):
            # PE warm-up: tiny matmuls on memset tiles, queued ahead of the
            # real ones so the clock is ramped when data arrives.
            wrm_w = cpool.tile([1, 1], BF16)
            ones_r = cpool.tile([1, MM_N], BF16)
            nc.vector.memset(wrm_w[:, :], 0.0)
            nc.vector.memset(ones_r[:, :], 1.0)
            for _ in range(N_WARM):
                pw = ppool.tile([O, MM_N], F32)
                nc.tensor.matmul(
                    pw[0:1, :], wrm_w[:, :], ones_r[:, :], start=True, stop=True
                )

            wt = cpool.tile([128, 384], BF16)
            nc.sync.dma_start(out=wt[:, :], in_=w_h[:, :])
            bt = cpool.tile([O, 1], F32)
            nc.sync.dma_start(out=bt[:, :], in_=bias_h[:, :])
            bbt = cpool.tile([1, O], BF16)
            nc.sync.dma_start(out=bbt[:, :], in_=bb_h[:, :])

            xt = wpool.tile([128, FREE], BF16)
            pb = wpool.tile([128, FREE], BF16)
            yt = wpool.tile([128, FREE], BF16)
            st = wpool.tile([O, HS * W], F32)

            for j in range(len(EW_EDGES) - 1):
                c0, c1 = EW_EDGES[j], EW_EDGES[j + 1]
                sl = slice(c0, c1)
                su = slice(c0 + WP, c1 + WP)
                nc.sync.dma_start(out=xt[0:64, sl], in_=x_h[:, sl])
                nc.sync.dma_start(out=xt[64:128, sl], in_=x_h[:, su])
                nc.gpsimd.dma_start(
                    out=pb[0:64, sl], in_=bcast_ap(m_h[:, :], c0, c1 - c0)
                )
                nc.gpsimd.dma_start(
                    out=pb[64:128, sl], in_=bcast_ap(m_h[:, :], c0 + WP, c1 - c0)
                )
                nc.vector.tensor_mul(yt[:, sl], pb[:, sl], xt[:, sl])

            y3 = yt[:].rearrange("p (r c) -> p r c", r=RP)
            for i in range(NCHUNK):
                ps = ppool.tile([O, MM_N], F32)
                p3 = ps[:].rearrange("p (r c) -> p r c", r=4)
                for l in range(3):
                    # singles: tap k=2, lower half only
                    nc.tensor.matmul(
                        p3,
                        wt[0:64, 192 + 64 * l : 192 + 64 * l + 64],
                        y3[0:64, 4 * i + 2 : 4 * i + 6, l : l + W],
                        start=(l == 0),
                        stop=False,
                    )
                for l in range(3):
                    # paired taps k=0 (lower half) + k=1 (shifted half)
                    nc.tensor.matmul(
                        p3,
                        wt[0:128, 64 * l : 64 * l + 64],
                        y3[0:128, 4 * i : 4 * i + 4, l : l + W],
                        start=False,
                        stop=(l == 2),
                    )
                # bias-add while copying PSUM -> SBUF staging (ACT engine)
                nc.scalar.add(st[:, 512 * i : 512 * i + 512], ps[:, :], bt[:, 0:1])
            for g0, g1 in STORE_GROUPS:
                nc.sync.dma_start(
                    out=out_h[:, 512 * g0 : 512 * g1], in_=st[:, 512 * g0 : 512 * g1]
                )
    return nc


def _pack_weights(wt):
    """[O,C,3,3] -> [128, 384] bf16: cols l*64+o rows c|c = taps (0,l)|(1,l);
    cols 192+l*64+o rows c (lower 64) = tap (2,l)."""
    wk = wt.transpose(1, 2, 3, 0)  # [c, k, l, o]
    pair = np.concatenate([wk[:, 0], wk[:, 1]], axis=0).reshape(128, 192)
    single = wk[:, 2].reshape(64, 192)
    out = np.zeros((128, 384), np.float32)
    out[:, :192] = pair
    out[:64, 192:] = single
    return np.ascontiguousarray(out.astype(ml_dtypes.bfloat16))


def kernel(inputs, alpha, weight, bias, a, b, c):
    x = np.asarray(inputs, np.float32)
    al = np.asarray(alpha, np.float32)
    wt = np.asarray(weight, np.float32)
    bs = np.asarray(bias, np.float32)
    av, bv, cv = float(a), float(b), float(c)

    if "nc" not in _cache:
        nc_new = _program()
        nc_new.finalize()
        _cache["nc"] = nc_new
    nc = _cache["nc"]

    w_packed = _pack_weights(wt)
    b_packed = np.ascontiguousarray(bs.reshape(O, 1))

    in_maps = []
    for core in range(8):
        b_idx, hh = divmod(core, 2)
        r0 = hh * HS - 1  # global row of padded row 0
        xs = np.zeros((C, RP + 1, WP), np.float32)
        als = np.zeros((1, RP + 1, WP), np.float32)
        lo = max(0, r0)
        hi = min(H, r0 + RP)
        xs[:, lo - r0 : hi - r0, 1 : 1 + W] = x[b_idx, :, lo:hi, :]
        als[:, lo - r0 : hi - r0, 1 : 1 + W] = al[b_idx, :, lo:hi, :]
        # p = poly(alpha) on the padded field (p=c at padding; x=0 there)
        m = (av * als + bv) * als + cv
        in_maps.append(
            {
                "x": np.ascontiguousarray(
                    xs.reshape(C, FREE2).astype(ml_dtypes.bfloat16)
                ),
                "m": np.ascontiguousarray(
                    m.reshape(1, FREE2).astype(ml_dtypes.bfloat16)
                ),
                "w": w_packed,
                "bias": b_packed,
                "bb": np.ascontiguousarray(
                    bs.reshape(1, O).astype(ml_dtypes.bfloat16)
                ),
            }
        )

    res = run_bass_kernel_spmd(nc, in_maps, list(range(8)))

    out = np.empty((B, O, H, W), np.float32)
    for core in range(8):
        b_idx, hh = divmod(core, 2)
        out[b_idx, :, hh * HS : (hh + 1) * HS, :] = res.results[core]["out"].reshape(
            O, HS, W
        )
    return out


# revision 15
# speedup vs baseline: 1.0342x; 1.0259x over previous
"""ConvSquare Trainium2 kernel.

Math: out = conv2d_3x3(x * p, weight) + bias, stride 1, pad 1, where
p = (a*alpha + b)*alpha + c on the zero-padded alpha field. (x is
zero-padded, so border window positions contribute 0 regardless of p.)

Sharding: 8 cores = batch(4) x row-half(2). Each core computes a
[O=64, 64, 128] output slab from a zero-padded [C=64, 67, 130] slab
(67th row all-zero, backing the +1-row shifted copy).

Device pipeline per core (bf16 datapath, f32 accumulate/output):
  - x loaded twice from HBM: partitions 0-63 = rows 0-65, partitions
    64-127 = rows 1-66 (the +1-row shift baked in at load time - no
    SBUF->SBUF shift copy).
  - p field (host-precomputed tiny poly, 0.001% of FLOPs) broadcast
    from HBM to both halves with the same shift.
  - One DVE tensor_mul per chunk produces y AND shifted-y together
    ([128, n] op costs the same as [64, n]; bf16 gets the 2x mode).
  - 6 matmuls per 512-col output chunk: 3 paired taps (k=0,1) over the
    128-partition tile + 3 singles (k=2) on the lower half.
  - ACT engine adds bias while copying PSUM->SBUF staging; grouped
    SBUF->HBM stores.
  - A few tiny warm-up matmuls ramp the PE clock before real work.
"""

import sys

import numpy as np

sys.path.insert(0, "/opt/trn_rl_repo")

import ml_dtypes

import concourse.bass as bass
import concourse.mybir as mybir
from concourse.bass_utils import run_bass_kernel_spmd
from concourse.tile import TileContext

F32 = mybir.dt.float32
BF16 = mybir.dt.bfloat16

B, C, O, H, W = 4, 64, 64, 128, 128
HS = 64  # output rows per core
RP = HS + 2  # padded input rows (66)
WP = W + 2  # padded cols (130)
FREE = RP * WP  # 8580
FREE2 = (RP + 1) * WP  # 8710: one extra all-zero row for the shifted half
NCHUNK = 16  # matmul chunks (4 out rows each)
MM_N = 4 * W  # 512
# elementwise chunk edges (cols): small first chunk so PE starts early
EW_EDGES = [0, 780, 2080, 3380, 4680, 5980, 7280, 8580]
N_WARM = 7
# matmul accumulation groups: (start_row, n_rows); tail split small so the
# last copy+store chain after the final matmul is short
MM_CHUNKS = [(4 * i, 4) for i in range(15)] + [(60, 2), (62, 1), (63, 1)]
# SBUF->HBM store groups in staging-column units (out row r = cols 128r)
STORE_GROUPS = [
    (0, 3072),
    (3072, 5120),
    (5120, 6656),
    (6656, 7680),
    (7680, 7936),
    (7936, 8064),
    (8064, 8192),
]

_cache: dict = {}


def _program() -> bass.Bass:
    from concourse.bacc import Bacc

    nc = Bacc()
    x_h = nc.dram_tensor("x", [C, FREE2], BF16, kind="ExternalInput")
    m_h = nc.dram_tensor("m", [1, FREE2], BF16, kind="ExternalInput")
    w_h = nc.dram_tensor("w", [128, 384], BF16, kind="ExternalInput")
    bias_h = nc.dram_tensor("bias", [O, 1], F32, kind="ExternalInput")
    out_h = nc.dram_tensor("out", [O, HS * W], F32, kind="ExternalOutput")

    with TileContext(nc) as tc:
        with (
            tc.tile_pool(name="const", bufs=1) as cpool,
            tc.tile_pool(name="work", bufs=1) as wpool,
            tc.tile_pool(name="psum", bufs=4, space="PSUM") as ppool,
        ):
            # PE warm-up: tiny matmuls on memset tiles, queued ahead of the
            # real ones so the clock is ramped when data arrives.
            wrm_w = cpool.tile([1, 1], BF16)
            ones_r = cpool.tile([1, MM_N], BF16)
            nc.gpsimd.memset(wrm_w[:, :], 0.0)
            nc.vector.memset(ones_r[:, :], 1.0)
            for _ in range(N_WARM):
                pw = ppool.tile([O, MM_N], F32)
                nc.tensor.matmul(
                    pw[0:1, :], wrm_w[:, :], ones_r[:, :], start=True, stop=True
                )

            wt = cpool.tile([128, 384], BF16)
            bt = cpool.tile([O, 1], F32)
            xt = wpool.tile([128, FREE], BF16)
            pb = wpool.tile([128, FREE], BF16)
            yt = wpool.tile([128, FREE], BF16)
            st = wpool.tile([O, HS * W], F32)

            for j in range(len(EW_EDGES) - 1):
                c0, c1 = EW_EDGES[j], EW_EDGES[j + 1]
                n = c1 - c0
                sl = slice(c0, c1)
                # one DMA fills both halves: partitions 0-63 read offset c0,
                # partitions 64-127 read offset c0+WP (the +1-row shift)
                nc.sync.dma_start(
                    out=xt[:, sl],
                    in_=bass.AP(
                        tensor=x_h[:, :].tensor,
                        offset=c0,
                        ap=[[WP, 2], [FREE2, C], [1, n]],
                    ),
                )
                nc.gpsimd.dma_start(
                    out=pb[:, sl],
                    in_=bass.AP(
                        tensor=m_h[:, :].tensor,
                        offset=c0,
                        ap=[[WP, 2], [0, C], [1, n]],
                    ),
                )
                nc.vector.tensor_mul(yt[:, sl], pb[:, sl], xt[:, sl])
                if j == 0:
                    # small loads issued after the critical first chunk
                    nc.sync.dma_start(out=wt[:, :], in_=w_h[:, :])
                    nc.sync.dma_start(out=bt[:, :], in_=bias_h[:, :])

            y3 = yt[:].rearrange("p (r c) -> p r c", r=RP)
            for R, r in MM_CHUNKS:
                nf = r * W
                ps = ppool.tile([O, nf], F32)
                p3 = ps[:].rearrange("p (r c) -> p r c", r=r)
                for l in range(3):
                    # singles: tap k=2, lower half only
                    nc.tensor.matmul(
                        p3,
                        wt[0:64, 192 + 64 * l : 192 + 64 * l + 64],
                        y3[0:64, R + 2 : R + r + 2, l : l + W],
                        start=(l == 0),
                        stop=False,
                    )
                for l in range(3):
                    # paired taps k=0 (lower half) + k=1 (shifted half)
                    nc.tensor.matmul(
                        p3,
                        wt[0:128, 64 * l : 64 * l + 64],
                        y3[0:128, R : R + r, l : l + W],
                        start=False,
                        stop=(l == 2),
                    )
                # bias-add while copying PSUM -> SBUF staging (ACT engine)
                nc.scalar.add(st[:, W * R : W * (R + r)], ps[:, :], bt[:, 0:1])
            for g0, g1 in STORE_GROUPS:
                nc.sync.dma_start(
                    out=out_h[:, g0:g1], in_=st[:, g0:g1]
                )
    return nc


def _pack_weights(wt):
    """[O,C,3,3] -> [128, 384] bf16: cols l*64+o rows c|c = taps (0,l)|(1,l);
    cols 192+l*64+o rows c (lower 64) = tap (2,l)."""
    wk = wt.transpose(1, 2, 3, 0)  # [c, k, l, o]
    pair = np.concatenate([wk[:, 0], wk[:, 1]], axis=0).reshape(128, 192)
    single = wk[:, 2].reshape(64, 192)
    out = np.zeros((128, 384), np.float32)
    out[:, :192] = pair
    out[:64, 192:] = single
    return np.ascontiguousarray(out.astype(ml_dtypes.bfloat16))


def kernel(inputs, alpha, weight, bias, a, b, c):
    x = np.asarray(inputs, np.float32)
    al = np.asarray(alpha, np.float32)
    wt = np.asarray(weight, np.float32)
    bs = np.asarray(bias, np.float32)
    av, bv, cv = float(a), float(b), float(c)

    if "nc" not in _cache:
        nc_new = _program()
        nc_new.finalize()
        _cache["nc"] = nc_new
    nc = _cache["nc"]

    w_packed = _pack_weights(wt)
    b_packed = np.ascontiguousarray(bs.reshape(O, 1))

    in_maps = []
    for core in range(8):
        b_idx, hh = divmod(core, 2)
        r0 = hh * HS - 1  # global row of padded row 0
        xs = np.zeros((C, RP + 1, WP), np.float32)
        als = np.zeros((1, RP + 1, WP), np.float32)
        lo = max(0, r0)
        hi = min(H, r0 + RP)
        xs[:, lo - r0 : hi - r0, 1 : 1 + W] = x[b_idx, :, lo:hi, :]
        als[:, lo - r0 : hi - r0, 1 : 1 + W] = al[b_idx, :, lo:hi, :]
        # p = poly(alpha) on the padded field (p=c at padding; x=0 there)
        m = (av * als + bv) * als + cv
        in_maps.append(
            {
                "x": np.ascontiguousarray(
                    xs.reshape(C, FREE2).astype(ml_dtypes.bfloat16)
                ),
                "m": np.ascontiguousarray(
                    m.reshape(1, FREE2).astype(ml_dtypes.bfloat16)
                ),
                "w": w_packed,
                "bias": b_packed,
            }
        )

    res = run_bass_kernel_spmd(nc, in_maps, list(range(8)))

    out = np.empty((B, O, H, W), np.float32)
    for core in range(8):
        b_idx, hh = divmod(core, 2)
        out[b_idx, :, hh * HS : (hh + 1) * HS, :] = res.results[core]["out"].reshape(
            O, HS, W
        )
    return out


# revision 17
# speedup vs baseline: 1.0495x; 1.0148x over previous
"""ConvSquare Trainium2 kernel.

Math: out = conv2d_3x3(x * p, weight) + bias, stride 1, pad 1, where
p = (a*alpha + b)*alpha + c on the zero-padded alpha field. (x is
zero-padded, so border window positions contribute 0 regardless of p.)

Sharding: 8 cores = batch(4) x row-half(2). Each core computes a
[O=64, 64, 128] output slab from a zero-padded [C=64, 67, 130] slab
(67th row all-zero, backing the +1-row shifted copy).

Device pipeline per core (bf16 datapath, f32 accumulate/output):
  - x loaded twice from HBM: partitions 0-63 = rows 0-65, partitions
    64-127 = rows 1-66 (the +1-row shift baked in at load time - no
    SBUF->SBUF shift copy).
  - p field (host-precomputed tiny poly, 0.001% of FLOPs) broadcast
    from HBM to both halves with the same shift.
  - One DVE tensor_mul per chunk produces y AND shifted-y together
    ([128, n] op costs the same as [64, n]; bf16 gets the 2x mode).
  - 6 matmuls per 512-col output chunk: 3 paired taps (k=0,1) over the
    128-partition tile + 3 singles (k=2) on the lower half.
  - ACT engine adds bias while copying PSUM->SBUF staging; grouped
    SBUF->HBM stores.
  - A few tiny warm-up matmuls ramp the PE clock before real work.
"""

import sys

import numpy as np

sys.path.insert(0, "/opt/trn_rl_repo")

import ml_dtypes

import concourse.bass as bass
import concourse.mybir as mybir
from concourse.bass_utils import run_bass_kernel_spmd
from concourse.tile import TileContext

F32 = mybir.dt.float32
BF16 = mybir.dt.bfloat16

B, C, O, H, W = 4, 64, 64, 128, 128
HS = 64  # output rows per core
RP = HS + 2  # padded input rows (66)
WP = W + 2  # padded cols (130)
FREE = RP * WP  # 8580
FREE2 = (RP + 1) * WP  # 8710: one extra all-zero row for the shifted half
NCHUNK = 16  # matmul chunks (4 out rows each)
MM_N = 4 * W  # 512
# elementwise chunk edges (cols): small early chunks so PE starts early
EW_EDGES = [0, 520, 1040, 1820, 2860, 4160, 5460, 6760, 7930, 8580]
N_WARM = 5
# matmul accumulation groups: (start_row, n_rows); small groups at the start
# (early PE launch) and at the end (short final copy+store chain)
MM_CHUNKS = (
    [(0, 2), (2, 2), (4, 2), (6, 2)]
    + [(8 + 4 * i, 4) for i in range(13)]
    + [(60, 2), (62, 1), (63, 1)]
)
# SBUF->HBM store groups in staging-column units (out row r = cols 128r)
STORE_GROUPS = [
    (0, 3072),
    (3072, 5120),
    (5120, 6656),
    (6656, 7680),
    (7680, 7936),
]

_cache: dict = {}


def _program() -> bass.Bass:
    from concourse.bacc import Bacc

    nc = Bacc()
    x_h = nc.dram_tensor("x", [C, FREE2], BF16, kind="ExternalInput")
    m_h = nc.dram_tensor("m", [1, FREE2], BF16, kind="ExternalInput")
    w_h = nc.dram_tensor("w", [128, 384], BF16, kind="ExternalInput")
    bias_h = nc.dram_tensor("bias", [O, 1], F32, kind="ExternalInput")
    out_h = nc.dram_tensor("out", [O, HS * W], F32, kind="ExternalOutput")

    with TileContext(nc) as tc:
        with (
            tc.tile_pool(name="const", bufs=1) as cpool,
            tc.tile_pool(name="work", bufs=1) as wpool,
            tc.tile_pool(name="psum", bufs=4, space="PSUM") as ppool,
        ):
            # PE warm-up: tiny matmuls on memset tiles, queued ahead of the
            # real ones so the clock is ramped when data arrives.
            wrm_w = cpool.tile([1, 1], BF16)
            ones_r = cpool.tile([1, MM_N], BF16)
            nc.gpsimd.memset(wrm_w[:, :], 0.0)
            nc.vector.memset(ones_r[:, :], 1.0)
            for _ in range(N_WARM):
                pw = ppool.tile([O, MM_N], F32)
                nc.tensor.matmul(
                    pw[0:1, :], wrm_w[:, :], ones_r[:, :], start=True, stop=True
                )

            wt = cpool.tile([128, 384], BF16)
            bt = cpool.tile([O, 1], F32)
            xt = wpool.tile([128, FREE], BF16)
            pb = wpool.tile([128, FREE], BF16)
            yt = wpool.tile([128, FREE], BF16)
            st = wpool.tile([O, HS * W], F32)

            for j in range(len(EW_EDGES) - 1):
                c0, c1 = EW_EDGES[j], EW_EDGES[j + 1]
                n = c1 - c0
                sl = slice(c0, c1)
                # one DMA fills both halves: partitions 0-63 read offset c0,
                # partitions 64-127 read offset c0+WP (the +1-row shift)
                nc.sync.dma_start(
                    out=xt[:, sl],
                    in_=bass.AP(
                        tensor=x_h[:, :].tensor,
                        offset=c0,
                        ap=[[WP, 2], [FREE2, C], [1, n]],
                    ),
                )
                nc.gpsimd.dma_start(
                    out=pb[:, sl],
                    in_=bass.AP(
                        tensor=m_h[:, :].tensor,
                        offset=c0,
                        ap=[[WP, 2], [0, C], [1, n]],
                    ),
                )
                nc.vector.tensor_mul(yt[:, sl], pb[:, sl], xt[:, sl])
                if j == 0:
                    # small loads issued after the critical first chunk
                    nc.sync.dma_start(out=wt[:, :], in_=w_h[:, :])
                    nc.sync.dma_start(out=bt[:, :], in_=bias_h[:, :])

            y3 = yt[:].rearrange("p (r c) -> p r c", r=RP)
            for R, r in MM_CHUNKS:
                nf = r * W
                ps = ppool.tile([O, nf], F32)
                p3 = ps[:].rearrange("p (r c) -> p r c", r=r)
                for l in range(3):
                    # singles: tap k=2, lower half only
                    nc.tensor.matmul(
                        p3,
                        wt[0:64, 192 + 64 * l : 192 + 64 * l + 64],
                        y3[0:64, R + 2 : R + r + 2, l : l + W],
                        start=(l == 0),
                        stop=False,
                    )
                for l in range(3):
                    # paired taps k=0 (lower half) + k=1 (shifted half)
                    nc.tensor.matmul(
                        p3,
                        wt[0:128, 64 * l : 64 * l + 64],
                        y3[0:128, R : R + r, l : l + W],
                        start=False,
                        stop=(l == 2),
                    )
                # bias-add while copying PSUM -> SBUF staging; the last two
                # tail pieces go to DVE/ACT in parallel to shorten the tail
                ss = st[:, W * R : W * (R + r)]
                if R == 62:
                    nc.vector.tensor_scalar(
                        out=ss, in0=ps[:, :], scalar1=bt[:, 0:1], scalar2=None,
                        op0=mybir.AluOpType.add,
                    )
                else:
                    nc.scalar.add(ss, ps[:, :], bt[:, 0:1])
            for g0, g1 in STORE_GROUPS:
                nc.sync.dma_start(out=out_h[:, g0:g1], in_=st[:, g0:g1])
            # tail stores on three different queues so their issue overlaps
            nc.sync.dma_start(out=out_h[:, 7936:8064], in_=st[:, 7936:8064])
            nc.scalar.dma_start(out=out_h[:, 8064:8192], in_=st[:, 8064:8192])
    return nc


def _pack_weights(wt):
    """[O,C,3,3] -> [128, 384] bf16: cols l*64+o rows c|c = taps (0,l)|(1,l);
    cols 192+l*64+o rows c (lower 64) = tap (2,l)."""
    wk = wt.transpose(1, 2, 3, 0)  # [c, k, l, o]
    pair = np.concatenate([wk[:, 0], wk[:, 1]], axis=0).reshape(128, 192)
    single = wk[:, 2].reshape(64, 192)
    out = np.zeros((128, 384), np.float32)
    out[:, :192] = pair
    out[:64, 192:] = single
    return np.ascontiguousarray(out.astype(ml_dtypes.bfloat16))


def kernel(inputs, alpha, weight, bias, a, b, c):
    x = np.asarray(inputs, np.float32)
    al = np.asarray(alpha, np.float32)
    wt = np.asarray(weight, np.float32)
    bs = np.asarray(bias, np.float32)
    av, bv, cv = float(a), float(b), float(c)

    if "nc" not in _cache:
        nc_new = _program()
        nc_new.finalize()
        _cache["nc"] = nc_new
    nc = _cache["nc"]

    w_packed = _pack_weights(wt)
    b_packed = np.ascontiguousarray(bs.reshape(O, 1))

    in_maps = []
    for core in range(8):
        b_idx, hh = divmod(core, 2)
        r0 = hh * HS - 1  # global row of padded row 0
        xs = np.zeros((C, RP + 1, WP), np.float32)
        als = np.zeros((1, RP + 1, WP), np.float32)
        lo = max(0, r0)
        hi = min(H, r0 + RP)
        xs[:, lo - r0 : hi - r0, 1 : 1 + W] = x[b_idx, :, lo:hi, :]
        als[:, lo - r0 : hi - r0, 1 : 1 + W] = al[b_idx, :, lo:hi, :]
        # p = poly(alpha) on the padded field (p=c at padding; x=0 there)
        m = (av * als + bv) * als + cv
        in_maps.append(
            {
                "x": np.ascontiguousarray(
                    xs.reshape(C, FREE2).astype(ml_dtypes.bfloat16)
                ),
                "m": np.ascontiguousarray(
                    m.reshape(1, FREE2).astype(ml_dtypes.bfloat16)
                ),
                "w": w_packed,
                "bias": b_packed,
            }
        )

    res = run_bass_kernel_spmd(nc, in_maps, list(range(8)))

    out = np.empty((B, O, H, W), np.float32)
    for core in range(8):
        b_idx, hh = divmod(core, 2)
        out[b_idx, :, hh * HS : (hh + 1) * HS, :] = res.results[core]["out"].reshape(
            O, HS, W
        )
    return out


# revision 19
# speedup vs baseline: 1.0740x; 1.0233x over previous
"""ConvSquare Trainium2 kernel.

Math: out = conv2d_3x3(x * p, weight) + bias, stride 1, pad 1, where
p = (a*alpha + b)*alpha + c on the zero-padded alpha field. (x is
zero-padded, so border window positions contribute 0 regardless of p.)

Sharding: 8 cores = batch(4) x row-half(2). Each core computes a
[O=64, 64, 128] output slab from a zero-padded [C=64, 67, 130] slab
(67th row all-zero, backing the +1-row shifted copy).

Device pipeline per core (bf16 datapath, f32 accumulate/output):
  - x loaded twice from HBM: partitions 0-63 = rows 0-65, partitions
    64-127 = rows 1-66 (the +1-row shift baked in at load time - no
    SBUF->SBUF shift copy).
  - p field (host-precomputed tiny poly, 0.001% of FLOPs) broadcast
    from HBM to both halves with the same shift.
  - One DVE tensor_mul per chunk produces y AND shifted-y together
    ([128, n] op costs the same as [64, n]; bf16 gets the 2x mode).
  - 6 matmuls per 512-col output chunk: 3 paired taps (k=0,1) over the
    128-partition tile + 3 singles (k=2) on the lower half.
  - ACT engine adds bias while copying PSUM->SBUF staging; grouped
    SBUF->HBM stores.
  - A few tiny warm-up matmuls ramp the PE clock before real work.
"""

import sys

import numpy as np

sys.path.insert(0, "/opt/trn_rl_repo")

import ml_dtypes

import concourse.bass as bass
import concourse.mybir as mybir
from concourse.bass_utils import run_bass_kernel_spmd
from concourse.tile import TileContext

F32 = mybir.dt.float32
BF16 = mybir.dt.bfloat16

B, C, O, H, W = 4, 64, 64, 128, 128
HS = 64  # output rows per core
RP = HS + 2  # padded input rows (66)
WP = W + 2  # padded cols (130)
FREE = RP * WP  # 8580
FREE2 = (RP + 1) * WP  # 8710: one extra all-zero row for the shifted half
NCHUNK = 16  # matmul chunks (4 out rows each)
MM_N = 4 * W  # 512
# elementwise chunk edges (cols): small early chunks so PE starts early and
# the per-chunk DMA-sem/TT latency pipeline stays ahead of PE consumption
EW_EDGES = [0, 520, 1040, 1560, 2080, 2600, 3380, 4420, 5460, 6500, 7540, 8580]
N_WARM = 5
# matmul accumulation groups: (start_row, n_rows); small groups at the start
# (early PE launch) and at the end (short final copy+store chain)
MM_CHUNKS = (
    [(0, 2), (2, 2), (4, 2), (6, 2)]
    + [(8 + 4 * i, 4) for i in range(13)]
    + [(60, 2), (62, 1), (63, 1)]
)
# SBUF->HBM store groups in staging-column units (out row r = cols 128r)
STORE_GROUPS = [
    (0, 3072),
    (3072, 5120),
    (5120, 6656),
    (6656, 7680),
    (7680, 7936),
]

_cache: dict = {}


def _program() -> bass.Bass:
    from concourse.bacc import Bacc

    nc = Bacc()
    x_h = nc.dram_tensor("x", [C, FREE2], BF16, kind="ExternalInput")
    m_h = nc.dram_tensor("m", [1, FREE2], BF16, kind="ExternalInput")
    w_h = nc.dram_tensor("w", [128, 384], BF16, kind="ExternalInput")
    bias_h = nc.dram_tensor("bias", [O, 1], F32, kind="ExternalInput")
    out_h = nc.dram_tensor("out", [O, HS * W], F32, kind="ExternalOutput")

    with TileContext(nc) as tc:
        with (
            tc.tile_pool(name="const", bufs=1) as cpool,
            tc.tile_pool(name="work", bufs=1) as wpool,
            tc.tile_pool(name="psum", bufs=4, space="PSUM") as ppool,
        ):
            # PE warm-up: tiny matmuls on memset tiles, queued ahead of the
            # real ones so the clock is ramped when data arrives.
            wrm_w = cpool.tile([1, 1], BF16)
            ones_r = cpool.tile([1, MM_N], BF16)
            nc.gpsimd.memset(wrm_w[:, :], 0.0)
            nc.vector.memset(ones_r[:, :], 1.0)
            for _ in range(N_WARM):
                pw = ppool.tile([O, MM_N], F32)
                nc.tensor.matmul(
                    pw[0:1, :], wrm_w[:, :], ones_r[:, :], start=True, stop=True
                )

            wt = cpool.tile([128, 384], BF16)
            bt = cpool.tile([O, 1], F32)
            xt = wpool.tile([128, FREE], BF16)
            pb = wpool.tile([128, FREE], BF16)
            yt = wpool.tile([128, FREE], BF16)
            st = wpool.tile([O, HS * W], F32)

            for j in range(len(EW_EDGES) - 1):
                c0, c1 = EW_EDGES[j], EW_EDGES[j + 1]
                n = c1 - c0
                sl = slice(c0, c1)
                # one DMA fills both halves: partitions 0-63 read offset c0,
                # partitions 64-127 read offset c0+WP (the +1-row shift)
                nc.sync.dma_start(
                    out=xt[:, sl],
                    in_=bass.AP(
                        tensor=x_h[:, :].tensor,
                        offset=c0,
                        ap=[[WP, 2], [FREE2, C], [1, n]],
                    ),
                )
                nc.gpsimd.dma_start(
                    out=pb[:, sl],
                    in_=bass.AP(
                        tensor=m_h[:, :].tensor,
                        offset=c0,
                        ap=[[WP, 2], [0, C], [1, n]],
                    ),
                )
                nc.vector.tensor_mul(yt[:, sl], pb[:, sl], xt[:, sl])
                if j == 0:
                    # small loads issued after the critical first chunk
                    nc.sync.dma_start(out=wt[:, :], in_=w_h[:, :])
                    nc.sync.dma_start(out=bt[:, :], in_=bias_h[:, :])

            y3 = yt[:].rearrange("p (r c) -> p r c", r=RP)
            for R, r in MM_CHUNKS:
                nf = r * W
                ps = ppool.tile([O, nf], F32)
                p3 = ps[:].rearrange("p (r c) -> p r c", r=r)
                for l in range(3):
                    # singles: tap k=2, lower half only
                    nc.tensor.matmul(
                        p3,
                        wt[0:64, 192 + 64 * l : 192 + 64 * l + 64],
                        y3[0:64, R + 2 : R + r + 2, l : l + W],
                        start=(l == 0),
                        stop=False,
                    )
                for l in range(3):
                    # paired taps k=0 (lower half) + k=1 (shifted half)
                    nc.tensor.matmul(
                        p3,
                        wt[0:128, 64 * l : 64 * l + 64],
                        y3[0:128, R : R + r, l : l + W],
                        start=False,
                        stop=(l == 2),
                    )
                # bias-add while copying PSUM -> SBUF staging; the last two
                # tail pieces go to DVE/ACT in parallel to shorten the tail
                ss = st[:, W * R : W * (R + r)]
                if R == 62:
                    nc.vector.tensor_scalar(
                        out=ss, in0=ps[:, :], scalar1=bt[:, 0:1], scalar2=None,
                        op0=mybir.AluOpType.add,
                    )
                else:
                    nc.scalar.add(ss, ps[:, :], bt[:, 0:1])
            for g0, g1 in STORE_GROUPS:
                nc.sync.dma_start(out=out_h[:, g0:g1], in_=st[:, g0:g1])
            # tail stores on three different queues so their issue overlaps
            nc.gpsimd.dma_start(out=out_h[:, 7936:8064], in_=st[:, 7936:8064])
            nc.scalar.dma_start(out=out_h[:, 8064:8192], in_=st[:, 8064:8192])
    return nc


def _pack_weights(wt):
    """[O,C,3,3] -> [128, 384] bf16: cols l*64+o rows c|c = taps (0,l)|(1,l);
    cols 192+l*64+o rows c (lower 64) = tap (2,l)."""
    wk = wt.transpose(1, 2, 3, 0)  # [c, k, l, o]
    pair = np.concatenate([wk[:, 0], wk[:, 1]], axis=0).reshape(128, 192)
    single = wk[:, 2].reshape(64, 192)
    out = np.zeros((128, 384), np.float32)
    out[:, :192] = pair
    out[:64, 192:] = single
    return np.ascontiguousarray(out.astype(ml_dtypes.bfloat16))


def kernel(inputs, alpha, weight, bias, a, b, c):
    x = np.asarray(inputs, np.float32)
    al = np.asarray(alpha, np.float32)
    wt = np.asarray(weight, np.float32)
    bs = np.asarray(bias, np.float32)
    av, bv, cv = float(a), float(b), float(c)

    if "nc" not in _cache:
        nc_new = _program()
        nc_new.finalize()
        _cache["nc"] = nc_new
    nc = _cache["nc"]

    w_packed = _pack_weights(wt)
    b_packed = np.ascontiguousarray(bs.reshape(O, 1))

    in_maps = []
    for core in range(8):
        b_idx, hh = divmod(core, 2)
        r0 = hh * HS - 1  # global row of padded row 0
        xs = np.zeros((C, RP + 1, WP), np.float32)
        als = np.zeros((1, RP + 1, WP), np.float32)
        lo = max(0, r0)
        hi = min(H, r0 + RP)
        xs[:, lo - r0 : hi - r0, 1 : 1 + W] = x[b_idx, :, lo:hi, :]
        als[:, lo - r0 : hi - r0, 1 : 1 + W] = al[b_idx, :, lo:hi, :]
        # p = poly(alpha) on the padded field (p=c at padding; x=0 there)
        m = (av * als + bv) * als + cv
        in_maps.append(
            {
                "x": np.ascontiguousarray(
                    xs.reshape(C, FREE2).astype(ml_dtypes.bfloat16)
                ),
                "m": np.ascontiguousarray(
                    m.reshape(1, FREE2).astype(ml_dtypes.bfloat16)
                ),
                "w": w_packed,
                "bias": b_packed,
            }
        )

    res = run_bass_kernel_spmd(nc, in_maps, list(range(8)))

    out = np.empty((B, O, H, W), np.float32)
    for core in range(8):
        b_idx, hh = divmod(core, 2)
        out[b_idx, :, hh * HS : (hh + 1) * HS, :] = res.results[core]["out"].reshape(
            O, HS, W
        )
    return out


# revision 23
# speedup vs baseline: 1.0882x; 1.0132x over previous
"""ConvSquare Trainium2 kernel.

Math: out = conv2d_3x3(x * p, weight) + bias, stride 1, pad 1, where
p = (a*alpha + b)*alpha + c on the zero-padded alpha field. (x is
zero-padded, so border window positions contribute 0 regardless of p.)

Sharding: 8 cores = batch(4) x row-half(2). Each core computes a
[O=64, 64, 128] output slab from a zero-padded [C=64, 67, 130] slab
(67th row all-zero, backing the +1-row shifted copy).

Device pipeline per core (bf16 datapath, f32 accumulate/output):
  - x loaded twice from HBM: partitions 0-63 = rows 0-65, partitions
    64-127 = rows 1-66 (the +1-row shift baked in at load time - no
    SBUF->SBUF shift copy).
  - p field (host-precomputed tiny poly, 0.001% of FLOPs) broadcast
    from HBM to both halves with the same shift.
  - One DVE tensor_mul per chunk produces y AND shifted-y together
    ([128, n] op costs the same as [64, n]; bf16 gets the 2x mode).
  - 6 matmuls per 512-col output chunk: 3 paired taps (k=0,1) over the
    128-partition tile + 3 singles (k=2) on the lower half.
  - ACT engine adds bias while copying PSUM->SBUF staging; grouped
    SBUF->HBM stores.
  - A few tiny warm-up matmuls ramp the PE clock before real work.
"""

import sys

import numpy as np

sys.path.insert(0, "/opt/trn_rl_repo")

import ml_dtypes

import concourse.bass as bass
import concourse.mybir as mybir
from concourse.bass_utils import run_bass_kernel_spmd
from concourse.tile import TileContext

F32 = mybir.dt.float32
BF16 = mybir.dt.bfloat16

B, C, O, H, W = 4, 64, 64, 128, 128
HS = 64  # output rows per core
RP = HS + 2  # padded input rows (66)
WP = W + 2  # padded cols (130)
FREE = RP * WP  # 8580
FREE2 = (RP + 1) * WP  # 8710: one extra all-zero row for the shifted half
NCHUNK = 16  # matmul chunks (4 out rows each)
MM_N = 4 * W  # 512
# elementwise chunk edges (cols): small early chunks so PE starts early and
# the per-chunk DMA-sem/TT latency pipeline stays ahead of PE consumption
EW_EDGES = [0, 520, 1040, 1560, 2080, 2600, 3380, 4420, 5460, 6500, 7540, 8580]
N_WARM = 5
# matmul accumulation groups: (start_row, n_rows); small groups at the start
# (early PE launch) and at the end (short final copy+store chain)
MM_CHUNKS = (
    [(0, 2), (2, 2), (4, 2), (6, 2)]
    + [(8 + 4 * i, 4) for i in range(13)]
    + [(60, 2), (62, 1), (63, 1)]
)
# SBUF->HBM store groups in staging-column units (out row r = cols 128r)
STORE_GROUPS = [
    (0, 3072),
    (3072, 5120),
    (5120, 6656),
    (6656, 7680),
    (7680, 7936),
]

_cache: dict = {}


def _program() -> bass.Bass:
    from concourse.bacc import Bacc

    nc = Bacc()
    # xm packs x and the 64x-replicated p field: row c = [x[c] | p]
    xm_h = nc.dram_tensor("xm", [C, 2 * FREE2], BF16, kind="ExternalInput")
    w_h = nc.dram_tensor("w", [128, 384], BF16, kind="ExternalInput")
    bias_h = nc.dram_tensor("bias", [O, 1], F32, kind="ExternalInput")
    out_h = nc.dram_tensor("out", [O, HS * W], F32, kind="ExternalOutput")

    with TileContext(nc) as tc:
        with (
            tc.tile_pool(name="const", bufs=1) as cpool,
            tc.tile_pool(name="work", bufs=1) as wpool,
            tc.tile_pool(name="psum", bufs=4, space="PSUM") as ppool,
        ):
            # PE warm-up: tiny matmuls on memset tiles, queued ahead of the
            # real ones so the clock is ramped when data arrives.
            wrm_w = cpool.tile([1, 1], BF16)
            ones_r = cpool.tile([1, MM_N], BF16)
            nc.gpsimd.memset(wrm_w[:, :], 0.0)
            nc.vector.memset(ones_r[:, :], 1.0)
            for _ in range(N_WARM):
                pw = ppool.tile([O, MM_N], F32)
                nc.tensor.matmul(
                    pw[0:1, :], wrm_w[:, :], ones_r[:, :], start=True, stop=True
                )

            wt = cpool.tile([128, 384], BF16)
            bt = cpool.tile([O, 1], F32)
            # xp holds both operands: cols [0,FREE) = x, [FREE,2*FREE) = p,
            # partitions 64-127 = the +1-row-shifted copies of each
            xp = wpool.tile([128, 2 * FREE], BF16)
            yt = wpool.tile([128, FREE], BF16)
            st = wpool.tile([O, HS * W], F32)

            # weights/bias on the ACT queue so they never block x chunks
            nc.scalar.dma_start(out=wt[:, :], in_=w_h[:, :])
            nc.scalar.dma_start(out=bt[:, :], in_=bias_h[:, :])

            xp3 = xp[:].rearrange("p (s c) -> p s c", s=2)
            for j in range(len(EW_EDGES) - 1):
                c0, c1 = EW_EDGES[j], EW_EDGES[j + 1]
                n = c1 - c0
                # ONE DMA per chunk: iterates (shift h, channel c, sect s, e):
                # out col = s*FREE + c0 + e on partition h*64+c;
                # in flat = c*2*FREE2 + h*WP + s*FREE2 + c0 + e
                nc.sync.dma_start(
                    out=xp3[0:128, 0:2, c0:c1],
                    in_=bass.AP(
                        tensor=xm_h[:, :].tensor,
                        offset=c0,
                        ap=[[WP, 2], [2 * FREE2, C], [FREE2, 2], [1, n]],
                    ),
                )
                nc.vector.tensor_mul(
                    yt[:, c0:c1], xp[:, c0:c1], xp[:, FREE + c0 : FREE + c1]
                )

            y3 = yt[:].rearrange("p (r c) -> p r c", r=RP)
            for R, r in MM_CHUNKS:
                nf = r * W
                ps = ppool.tile([O, nf], F32)
                p3 = ps[:].rearrange("p (r c) -> p r c", r=r)
                for l in range(3):
                    # singles: tap k=2, lower half only
                    nc.tensor.matmul(
                        p3,
                        wt[0:64, 192 + 64 * l : 192 + 64 * l + 64],
                        y3[0:64, R + 2 : R + r + 2, l : l + W],
                        start=(l == 0),
                        stop=False,
                    )
                for l in range(3):
                    # paired taps k=0 (lower half) + k=1 (shifted half)
                    nc.tensor.matmul(
                        p3,
                        wt[0:128, 64 * l : 64 * l + 64],
                        y3[0:128, R : R + r, l : l + W],
                        start=False,
                        stop=(l == 2),
                    )
                # bias-add while copying PSUM -> SBUF staging; the last two
                # tail pieces go to DVE/ACT in parallel to shorten the tail
                ss = st[:, W * R : W * (R + r)]
                if R == 62:
                    nc.vector.tensor_scalar(
                        out=ss, in0=ps[:, :], scalar1=bt[:, 0:1], scalar2=None,
                        op0=mybir.AluOpType.add,
                    )
                else:
                    nc.scalar.add(ss, ps[:, :], bt[:, 0:1])
            for g0, g1 in STORE_GROUPS:
                nc.sync.dma_start(out=out_h[:, g0:g1], in_=st[:, g0:g1])
            # tail stores on three different queues so their issue overlaps
            nc.gpsimd.dma_start(out=out_h[:, 7936:8064], in_=st[:, 7936:8064])
            nc.scalar.dma_start(out=out_h[:, 8064:8192], in_=st[:, 8064:8192])
    return nc


def _pack_weights(wt):
    """[O,C,3,3] -> [128, 384] bf16: cols l*64+o rows c|c = taps (0,l)|(1,l);
    cols 192+l*64+o rows c (lower 64) = tap (2,l)."""
    wk = wt.transpose(1, 2, 3, 0)  # [c, k, l, o]
    pair = np.concatenate([wk[:, 0], wk[:, 1]], axis=0).reshape(128, 192)
    single = wk[:, 2].reshape(64, 192)
    out = np.zeros((128, 384), np.float32)
    out[:, :192] = pair
    out[:64, 192:] = single
    return np.ascontiguousarray(out.astype(ml_dtypes.bfloat16))


def kernel(inputs, alpha, weight, bias, a, b, c):
    x = np.asarray(inputs, np.float32)
    al = np.asarray(alpha, np.float32)
    wt = np.asarray(weight, np.float32)
    bs = np.asarray(bias, np.float32)
    av, bv, cv = float(a), float(b), float(c)

    if "nc" not in _cache:
        nc_new = _program()
        nc_new.finalize()
        _cache["nc"] = nc_new
    nc = _cache["nc"]

    w_packed = _pack_weights(wt)
    b_packed = np.ascontiguousarray(bs.reshape(O, 1))

    in_maps = []
    for core in range(8):
        b_idx, hh = divmod(core, 2)
        r0 = hh * HS - 1  # global row of padded row 0
        xs = np.zeros((C, RP + 1, WP), np.float32)
        als = np.zeros((1, RP + 1, WP), np.float32)
        lo = max(0, r0)
        hi = min(H, r0 + RP)
        xs[:, lo - r0 : hi - r0, 1 : 1 + W] = x[b_idx, :, lo:hi, :]
        als[:, lo - r0 : hi - r0, 1 : 1 + W] = al[b_idx, :, lo:hi, :]
        # p = poly(alpha) on the padded field (p=c at padding; x=0 there)
        m = ((av * als + bv) * als + cv).reshape(1, FREE2)
        xm = np.empty((C, 2 * FREE2), np.float32)
        xm[:, :FREE2] = xs.reshape(C, FREE2)
        xm[:, FREE2:] = m  # broadcast p to every channel row
        in_maps.append(
            {
                "xm": np.ascontiguousarray(xm.astype(ml_dtypes.bfloat16)),
                "w": w_packed,
                "bias": b_packed,
            }
        )

    res = run_bass_kernel_spmd(nc, in_maps, list(range(8)))

    out = np.empty((B, O, H, W), np.float32)
    for core in range(8):
        b_idx, hh = divmod(core, 2)
        out[b_idx, :, hh * HS : (hh + 1) * HS, :] = res.results[core]["out"].reshape(
            O, HS, W
        )
    return out
